# revision 23
# baseline (speedup 1.0000x reference)
"""BitwiseMLP Trainium2 kernel: 8-way data-parallel over the batch dim.

Math (per reference):
  h0 = x @ W0.T + b0; h0 = BN0(h0); s0 = sign(h0)
  h1 = s0 @ sign(W1).T + b1; h1 = BN1(h1); s1 = sign(h1)
  out = (s1 @ sign(W2).T + b2) * out_scale

Device strategy (per core, batch shard of 8192 rows; activations stay
transposed [channel, batch] end-to-end so the device does zero transposes):
  - L0 = x@W0.T needs ~15-bit-accurate products (a sign flip in s0 cascades
    through the two exact +-1 layers). Split: x = xh + dx (12-bit xh, read
    exactly through the PE's float32r/fp22 path), W0 = Wh + dW (fp16 Wh —
    stationary fp16 loads 2 elem/cycle so LDWEIGHTS fully hides; f32r
    stationary measured +17ns/MM):
      h0 = xh@Wh  (f32r moving x fp16 stationary W, 64 MMs/tile)
         + dx8@W8 + x8@dW8  (fp8e4m3 DoubleRow, both corrections sharing the
           DR pair slots: 64 DR MMs/tile, 2x MAC rate)
    All operands pre-scaled by powers of two so every product carries 2^25
    (main: xh*2^13 @ Wh*2^12; corrections: dx*2^17 @ W*2^8 and
    x*2^5 @ dW*2^20 — keeps fp8 operands inside e4m3's [2^-9, 240] range);
    the 2^-25 is folded into the BN0 scale of the sign activation.
    Bit-exact host sim: final rel err ~1.4e-2 (gate 2e-2).
  - Each batch tile runs phase-split: all 64 f32r mains, then all 64 DR
    corrections (one f32r->fp8 transition per tile instead of 16).
  - BN+sign fuse into one ScalarE activation per tile:
    s = Sign(psum * A + B) with per-channel A/B, output fp8e4 (+-1 exact).
  - L1/L2 are exact +-1 fp8e4 matmuls with DoubleRow (2x rate); results are
    small even integers accumulated exactly in fp32 PSUM.
  - Final eviction: Identity activation out = psum*out_scale + b2*out_scale.
  - Startup: weight DMAs split per block and deadline-ordered behind the
    first two batch-tiles' x DMAs; last batch tile runs as two 256-col
    halves to shorten the ScalarE/DMA tail behind the final matmul.
Host does the batch shard, the transposes and the hi/lo splits; the output
comes back transposed per core and is re-assembled in numpy.
"""
import os
import sys
import types

import numpy as np
import ml_dtypes

import concourse.bass as bass
import concourse.mybir as mybir
import concourse.tile as tile
from concourse import bacc
from concourse.bass_utils import run_bass_kernel_spmd


def _ensure_axon_hooks():
    """concourse.bass_utils imports antenv.axon_hooks when tracing is
    requested (BASS_TRACE=1). The trimmed image lacks that module, which
    would turn an optional profile into a crash — synthesize it, wiring the
    real NTFF hook when libaxon_pjrt.so is present."""
    try:
        import antenv.axon_hooks  # noqa: F401
        return
    except ImportError:
        pass
    try:
        import antenv
    except ImportError:
        return
    mod = types.ModuleType("antenv.axon_hooks")
    state = {"hook": None}
    mod.set_axon_ntff_profile_hook = lambda h: state.update(hook=h)
    mod.get_axon_ntff_profile_hook = lambda: state["hook"]
    sys.modules["antenv.axon_hooks"] = mod
    antenv.axon_hooks = mod
    so = "/opt/axon/libaxon_pjrt.so"
    if os.path.exists(so):
        try:
            from trn_agent_boot.trn_boot import _ntff_profile_via_ctypes
            mod.set_axon_ntff_profile_hook(_ntff_profile_via_ctypes(so))
            import concourse.bass_utils as _bu
            _real_upload = _bu.upload_artifacts

            def _safe_upload(tmpdir):
                try:
                    return _real_upload(tmpdir)
                except Exception:
                    return f"local:{tmpdir}"

            _bu.upload_artifacts = _safe_upload
        except Exception:
            pass


_ensure_axon_hooks()

dt = mybir.dt
P = 128
D = 1024
B = 65536
NCORES = 8
BS = B // NCORES          # 8192 batch rows per core
BT = 512                  # batch-tile width (columns of transposed activations)
NBT = BS // BT            # 16 batch tiles per core
KO = D // P               # 8 k-subtiles of 128 channels
EPS = 1e-5

# NOTE: neuronxcc rejects mixed 32-bit/non-32-bit matmul inputs, so the
# main is either f32r x @ f32r W (12-bit operands, ~221.6ns/MM cadence,
# HW rel err 1.137e-2) or fp16 x @ fp16 W (11-bit operands, ~208ns/MM,
# LDWEIGHTS fully hidden, sim rel err 1.65e-2 — both under the 2e-2 gate).
USE_FP16_MAIN = True

# power-of-two operand pre-scales; every product carries 2^25
SC_XH = np.float32(2.0 ** 13)   # main, x side (moving)
SC_WH = np.float32(2.0 ** 12)   # main, W side (stationary)
if USE_FP16_MAIN:
    SC_DX = np.float32(2.0 ** 16)   # corrX: dx8 = e4m3(dx * 2^16) (dx ~ 2^-11 x)
    SC_W8 = np.float32(2.0 ** 9)    # corrX: W8  = e4m3(W * 2^9)
else:
    SC_DX = np.float32(2.0 ** 17)   # corrX: dx8 = e4m3(dx * 2^17) (dx ~ 2^-12 x)
    SC_W8 = np.float32(2.0 ** 8)    # corrX: W8  = e4m3(W * 2^8)
SC_X8 = np.float32(2.0 ** 5)    # fp8 corrW: x8  = e4m3(x * 2^5)
SC_DW = np.float32(2.0 ** 20)   # fp8 corrW: dW8 = e4m3(dW * 2^20)
INV_PROD = np.float32(2.0 ** -25)

LAST_RESULTS = None       # BassKernelResults of the most recent run (for profiling)
_NC = None                # cached compiled Bass module (build once per process)


def _round_sig12(a: np.ndarray) -> np.ndarray:
    """Round fp32 magnitudes to 12-bit significands (11 explicit mantissa
    bits), round-half-to-even. Values of this form pass through the PE's
    float32r (fp22) operand read exactly."""
    u = a.view(np.uint32).astype(np.uint64)
    half = np.uint64(1 << 11)
    one = np.uint64(1)
    r = (u + half - one + ((u >> np.uint64(12)) & one)) & ~np.uint64((1 << 12) - 1)
    return r.astype(np.uint32).view(np.float32)


def _build():
    nc = bacc.Bacc(num_devices=NCORES)
    wdt = dt.float16 if USE_FP16_MAIN else dt.float32r
    xdt = dt.float16 if USE_FP16_MAIN else dt.float32r
    xh = nc.dram_tensor("xh", [P, KO, BS], xdt, kind="ExternalInput")
    xc = nc.dram_tensor("xc", [P, KO, 2, BS], dt.float8e4, kind="ExternalInput")
    w0h = nc.dram_tensor("w0h", [P, KO, D], wdt, kind="ExternalInput")
    wc = nc.dram_tensor("wc", [P, KO, 2, D], dt.float8e4, kind="ExternalInput")
    w1 = nc.dram_tensor("w1", [P, KO, D], dt.float8e4, kind="ExternalInput")
    w2 = nc.dram_tensor("w2", [P, KO, D], dt.float8e4, kind="ExternalInput")
    vec = nc.dram_tensor("vec", [P, 6, KO], dt.float32, kind="ExternalInput")
    out = nc.dram_tensor("out", [P, KO, BS], dt.float32, kind="ExternalOutput")

    Sign = mybir.ActivationFunctionType.Sign
    Ident = mybir.ActivationFunctionType.Identity
    DR = mybir.MatmulPerfMode.DoubleRow
    ts = bass.ts

    with tile.TileContext(nc) as tc:
        with (
            tc.tile_pool(name="wpool", bufs=1) as wpool,
            tc.tile_pool(name="xpool", bufs=2) as xpool,
            tc.tile_pool(name="spool", bufs=2) as spool,
            tc.tile_pool(name="opool", bufs=4) as opool,
            tc.tile_pool(name="pspool", bufs=8, space="PSUM") as pspool,
        ):
            w0h_sb = wpool.tile([P, KO, D], wdt)
            wc_sb = wpool.tile([P, KO, 2, D], dt.float8e4)
            w1_sb = wpool.tile([P, KO, D], dt.float8e4)
            w2_sb = wpool.tile([P, KO, D], dt.float8e4)
            vec_sb = wpool.tile([P, 6, KO], dt.float32)

            xh_t, xc_t = xh[:], xc[:]
            w0h_t, wc_t = w0h[:], wc[:]
            out_t = out[:]

            # deadline-ordered DMA (single queue, ~220GB/s early): bt0 x,
            # main weights (per m-block), BN vector, bt0 corrections,
            # correction weights, L1/L2 weights (needed ~55/62us), then the
            # bt1 prefetch.
            x_tiles = []
            for bt in (0, 1):
                sl = bass.ds(bt * BT, BT)
                xh_sb = xpool.tile([P, KO, BT], xdt, tag="xh")
                xc_sb = xpool.tile([P, KO, 2, BT], dt.float8e4, tag="xc")
                x_tiles.append((xh_sb, xc_sb))
                if bt == 0:
                    nc.sync.dma_start(xh_sb, xh_t[:, :, sl])
                    for m in range(KO):
                        msl = ts(m, P)
                        nc.sync.dma_start(w0h_sb[:, :, msl], w0h_t[:, :, msl])
                    nc.sync.dma_start(vec_sb, vec[:])
                    nc.sync.dma_start(xc_sb, xc_t[:, :, :, sl])
                    for m in range(KO):
                        msl = ts(m, P)
                        nc.sync.dma_start(wc_sb[:, :, :, msl], wc_t[:, :, :, msl])
                    nc.sync.dma_start(w1_sb, w1[:])
                    nc.sync.dma_start(w2_sb, w2[:])
                else:
                    nc.sync.dma_start(xh_sb, xh_t[:, :, sl])
                    nc.sync.dma_start(xc_sb, xc_t[:, :, :, sl])

            def emit_tile(xh_sb, xc_sb, bt, c0, cols, sync_out=False):
                """One batch tile's compute: phase-split L0, then L1, L2.
                c0/cols select a column range within the SBUF x tiles (the
                last batch tile runs as two halves)."""
                cs = bass.ds(c0, cols)
                osl = bass.ds(bt * BT + c0, cols)
                # L0 in two groups of 4 m-blocks: all f32r mains of the
                # group, then its fp8-DR corrections — the f32r->fp8 PE
                # transition costs ~165ns, so batching same-dtype matmuls
                # pays, while holding only 4 PSUM banks keeps slack.
                s0_sb = spool.tile([P, KO, BT], dt.float8e4, tag="s0")
                GRP = 1
                for g in range(0, KO, GRP):
                    ps_g = [pspool.tile([P, BT], dt.float32, tag="ps",
                                        name=f"ps0_{g + i}")
                            for i in range(GRP)]
                    for i in range(GRP):
                        for k in range(KO):
                            nc.tensor.matmul(ps_g[i][:, :cols],
                                             w0h_sb[:, k, ts(g + i, P)],
                                             xh_sb[:, k, cs],
                                             start=k == 0, stop=False)
                    for i in range(GRP):
                        m = g + i
                        for k in range(KO):
                            nc.tensor.matmul(ps_g[i][:, :cols],
                                             wc_sb[:, k, :, ts(m, P)],
                                             xc_sb[:, k, :, cs],
                                             start=False, stop=k == KO - 1,
                                             perf_mode=DR)
                        nc.scalar.activation(s0_sb[:, m, cs], ps_g[i][:, :cols],
                                             Sign,
                                             bias=vec_sb[:, 1, m:m + 1],
                                             scale=vec_sb[:, 0, m:m + 1])
                # L1
                s1_sb = spool.tile([P, KO, BT], dt.float8e4, tag="s1")
                for m in range(KO):
                    ps = pspool.tile([P, BT], dt.float32, tag="ps")
                    for kp in range(KO // 2):
                        nc.tensor.matmul(ps[:, :cols],
                                         w1_sb[:, 2 * kp:2 * kp + 2, ts(m, P)],
                                         s0_sb[:, 2 * kp:2 * kp + 2, cs],
                                         start=kp == 0, stop=kp == KO // 2 - 1,
                                         perf_mode=DR)
                    nc.scalar.activation(s1_sb[:, m, cs], ps[:, :cols], Sign,
                                         bias=vec_sb[:, 3, m:m + 1],
                                         scale=vec_sb[:, 2, m:m + 1])
                # L2
                for m in range(KO):
                    ps = pspool.tile([P, BT], dt.float32, tag="ps")
                    for kp in range(KO // 2):
                        nc.tensor.matmul(ps[:, :cols],
                                         w2_sb[:, 2 * kp:2 * kp + 2, ts(m, P)],
                                         s1_sb[:, 2 * kp:2 * kp + 2, cs],
                                         start=kp == 0, stop=kp == KO // 2 - 1,
                                         perf_mode=DR)
                    o_sb = opool.tile([P, BT], dt.float32, tag="om")
                    nc.scalar.activation(o_sb[:, :cols], ps[:, :cols], Ident,
                                         bias=vec_sb[:, 5, m:m + 1],
                                         scale=vec_sb[:, 4, m:m + 1])
                    # out on the Scalar engine's DMA queue: keeps the Sync
                    # queue a pure input stream (no compute-dependent DMA
                    # blocking the next tile's prefetch behind it), and the
                    # in-order scalar queue sequences it right after its ACT.
                    # Final tile: Sync is idle (all inputs loaded), and its
                    # queue drains the tail without blocking the last ACTs.
                    eng = nc.sync if sync_out else nc.scalar
                    eng.dma_start(out_t[:, m, osl], o_sb[:, :cols])

            for bt in range(NBT):
                sl = bass.ds(bt * BT, BT)
                if bt < 2:
                    xh_sb, xc_sb = x_tiles[bt]
                else:
                    xh_sb = xpool.tile([P, KO, BT], xdt, tag="xh")
                    xc_sb = xpool.tile([P, KO, 2, BT], dt.float8e4, tag="xc")
                    nc.sync.dma_start(xh_sb, xh_t[:, :, sl])
                    nc.sync.dma_start(xc_sb, xc_t[:, :, :, sl])
                if bt == NBT - 1:
                    # split the final tile to shorten the post-matmul tail
                    emit_tile(xh_sb, xc_sb, bt, 0, BT // 2, sync_out=True)
                    emit_tile(xh_sb, xc_sb, bt, BT // 2, BT // 2,
                              sync_out=True)
                else:
                    emit_tile(xh_sb, xc_sb, bt, 0, BT)

    nc.compile()
    return nc


def kernel(**inputs) -> np.ndarray:
    global LAST_RESULTS
    f32 = np.float32
    e4m3 = mybir.dt.np(dt.float8e4)
    x = np.asarray(inputs["x"], f32)
    W0 = np.asarray(inputs["W0"], f32)
    b0 = np.asarray(inputs["b0"], f32)
    W1 = np.asarray(inputs["W1"], f32)
    b1 = np.asarray(inputs["b1"], f32)
    W2 = np.asarray(inputs["W2"], f32)
    b2 = np.asarray(inputs["b2"], f32)
    bn0_g = np.asarray(inputs["bn0_g"], f32)
    bn0_b = np.asarray(inputs["bn0_b"], f32)
    bn0_rm = np.asarray(inputs["bn0_rm"], f32)
    bn0_rv = np.asarray(inputs["bn0_rv"], f32)
    bn1_g = np.asarray(inputs["bn1_g"], f32)
    bn1_b = np.asarray(inputs["bn1_b"], f32)
    bn1_rm = np.asarray(inputs["bn1_rm"], f32)
    bn1_rv = np.asarray(inputs["bn1_rv"], f32)
    osc = np.asarray(inputs["out_scale"], f32)

    # per-channel affine folds (BN in eval mode):
    #   bn0(h+b0) = h*A0 + B0 ; bn1(h+b1) = h*A1 + B1 ; out = h*CS + CB
    # L0's psum carries h0*2^25, so A0 absorbs the 2^-25.
    inv0 = (bn0_g / np.sqrt(bn0_rv + EPS)).astype(f32)
    inv1 = (bn1_g / np.sqrt(bn1_rv + EPS)).astype(f32)
    A0 = (inv0 * INV_PROD).astype(f32)
    B0 = ((b0 - bn0_rm) * inv0 + bn0_b).astype(f32)
    A1, B1 = inv1, ((b1 - bn1_rm) * inv1 + bn1_b).astype(f32)
    CS, CB = osc, (b2 * osc).astype(f32)
    vec = np.stack([A0, B0, A1, B1, CS, CB])           # [6, D]
    vec_host = np.ascontiguousarray(
        vec.reshape(6, KO, P).transpose(2, 0, 1))      # [P, 6, KO]

    def pm(a):
        # [cols, D] -> partition-major [P, KO, cols]
        return np.ascontiguousarray(a.T.reshape(KO, P, -1).transpose(1, 0, 2))

    if USE_FP16_MAIN:
        Wh_sc = (W0 * SC_WH).astype(np.float16)        # stationary (2^12)
        dW = (W0 - Wh_sc.astype(f32) / SC_WH).astype(f32)
        w0h_host = pm(Wh_sc)
    else:
        W0h = _round_sig12(W0)
        dW = W0 - W0h
        w0h_host = pm(W0h * SC_WH)
    W8 = (W0 * SC_W8).astype(e4m3)
    dW8 = (dW * SC_DW).astype(e4m3)
    # correction weight pairs: [P, KO, 2, D]; slot0 = W8 (corrX, pairs with
    # dx8), slot1 = dW8 (corrW, pairs with x8)
    wc_host = np.ascontiguousarray(
        np.stack([pm(W8), pm(dW8)], axis=2))
    w1_host = pm(np.sign(W1).astype(e4m3))
    w2_host = pm(np.sign(W2).astype(e4m3))

    if USE_FP16_MAIN:
        xh_sc = (x * SC_XH).astype(np.float16)      # [*, D] fp16 (2^13)
        dx = (x - xh_sc.astype(f32) / SC_XH).astype(f32)
        xhT = pm(xh_sc)
    else:
        xh_full = _round_sig12(x)
        dx = x - xh_full
        xhT = pm(xh_full * SC_XH)                   # [P, KO, B] f32r (2^13)
    dx8 = (dx * SC_DX).astype(e4m3)
    x8 = (x * SC_X8).astype(e4m3)
    xcT = np.ascontiguousarray(
        np.stack([pm(dx8), pm(x8)], axis=2))        # [P, KO, 2, B]

    shared = {
        "w0h": w0h_host, "wc": wc_host,
        "w1": w1_host, "w2": w2_host, "vec": vec_host,
    }
    in_maps = []
    for c in range(NCORES):
        bs = slice(c * BS, (c + 1) * BS)
        in_maps.append({
            **shared,
            "xh": np.ascontiguousarray(xhT[:, :, bs]),
            "xc": np.ascontiguousarray(xcT[:, :, :, bs]),
        })

    global _NC
    if _NC is None:
        _NC = _build()
    res = run_bass_kernel_spmd(_NC, in_maps, core_ids=list(range(NCORES)))
    LAST_RESULTS = res

    out = np.empty((B, D), f32)
    for c in range(NCORES):
        # [P, KO, BS] -> [BS, KO*P] with channel = ko*P + p
        o = res.results[c]["out"].transpose(2, 1, 0).reshape(BS, D)
        out[c * BS:(c + 1) * BS] = o
    return out


# revision 24
# speedup vs baseline: 1.1917x; 1.1917x over previous
"""BitwiseMLP Trainium2 kernel: 8-way data-parallel over the batch dim.

Math (per reference):
  h0 = x @ W0.T + b0; h0 = BN0(h0); s0 = sign(h0)
  h1 = s0 @ sign(W1).T + b1; h1 = BN1(h1); s1 = sign(h1)
  out = (s1 @ sign(W2).T + b2) * out_scale

Device strategy (per core, batch shard of 8192 rows; activations stay
transposed [channel, batch] end-to-end so the device does zero transposes):
  - L0 = x@W0.T needs ~15-bit-accurate products (a sign flip in s0 cascades
    through the two exact +-1 layers). Split: x = xh + dx (12-bit xh, read
    exactly through the PE's float32r/fp22 path), W0 = Wh + dW (fp16 Wh —
    stationary fp16 loads 2 elem/cycle so LDWEIGHTS fully hides; f32r
    stationary measured +17ns/MM):
      h0 = xh@Wh  (f32r moving x fp16 stationary W, 64 MMs/tile)
         + dx8@W8 + x8@dW8  (fp8e4m3 DoubleRow, both corrections sharing the
           DR pair slots: 64 DR MMs/tile, 2x MAC rate)
    All operands pre-scaled by powers of two so every product carries 2^25
    (main: xh*2^13 @ Wh*2^12; corrections: dx*2^17 @ W*2^8 and
    x*2^5 @ dW*2^20 — keeps fp8 operands inside e4m3's [2^-9, 240] range);
    the 2^-25 is folded into the BN0 scale of the sign activation.
    Bit-exact host sim: final rel err ~1.4e-2 (gate 2e-2).
  - Each batch tile runs phase-split: all 64 f32r mains, then all 64 DR
    corrections (one f32r->fp8 transition per tile instead of 16).
  - BN+sign fuse into one ScalarE activation per tile:
    s = Sign(psum * A + B) with per-channel A/B, output fp8e4 (+-1 exact).
  - L1/L2 are exact +-1 fp8e4 matmuls with DoubleRow (2x rate); results are
    small even integers accumulated exactly in fp32 PSUM.
  - Final eviction: Identity activation out = psum*out_scale + b2*out_scale.
  - Startup: weight DMAs split per block and deadline-ordered behind the
    first two batch-tiles' x DMAs; last batch tile runs as two 256-col
    halves to shorten the ScalarE/DMA tail behind the final matmul.
Host does the batch shard, the transposes and the hi/lo splits; the output
comes back transposed per core and is re-assembled in numpy.
"""
import os
import sys
import types

import numpy as np
import ml_dtypes

import concourse.bass as bass
import concourse.mybir as mybir
import concourse.tile as tile
from concourse import bacc
from concourse.bass_utils import run_bass_kernel_spmd


def _ensure_axon_hooks():
    """concourse.bass_utils imports antenv.axon_hooks when tracing is
    requested (BASS_TRACE=1). The trimmed image lacks that module, which
    would turn an optional profile into a crash — synthesize it, wiring the
    real NTFF hook when libaxon_pjrt.so is present."""
    try:
        import antenv.axon_hooks  # noqa: F401
        return
    except ImportError:
        pass
    try:
        import antenv
    except ImportError:
        return
    mod = types.ModuleType("antenv.axon_hooks")
    state = {"hook": None}
    mod.set_axon_ntff_profile_hook = lambda h: state.update(hook=h)
    mod.get_axon_ntff_profile_hook = lambda: state["hook"]
    sys.modules["antenv.axon_hooks"] = mod
    antenv.axon_hooks = mod
    so = "/opt/axon/libaxon_pjrt.so"
    if os.path.exists(so):
        try:
            from trn_agent_boot.trn_boot import _ntff_profile_via_ctypes
            mod.set_axon_ntff_profile_hook(_ntff_profile_via_ctypes(so))
            import concourse.bass_utils as _bu
            _real_upload = _bu.upload_artifacts

            def _safe_upload(tmpdir):
                try:
                    return _real_upload(tmpdir)
                except Exception:
                    return f"local:{tmpdir}"

            _bu.upload_artifacts = _safe_upload
        except Exception:
            pass


_ensure_axon_hooks()

dt = mybir.dt
P = 128
D = 1024
B = 65536
NCORES = 8
BS = B // NCORES          # 8192 batch rows per core
BT = 512                  # batch-tile width (columns of transposed activations)
NBT = BS // BT            # 16 batch tiles per core
KO = D // P               # 8 k-subtiles of 128 channels
EPS = 1e-5

# NOTE: neuronxcc rejects mixed 32-bit/non-32-bit matmul inputs, so the
# main is either f32r x @ f32r W (12-bit operands, ~221.6ns/MM cadence,
# HW rel err 1.137e-2) or fp16 x @ fp16 W (11-bit operands, ~208ns/MM,
# LDWEIGHTS fully hidden, sim rel err 1.65e-2 — both under the 2e-2 gate).
USE_FP16_MAIN = True

# power-of-two operand pre-scales; every product carries 2^25
SC_XH = np.float32(2.0 ** 13)   # main, x side (moving)
SC_WH = np.float32(2.0 ** 12)   # main, W side (stationary)
if USE_FP16_MAIN:
    SC_DX = np.float32(2.0 ** 16)   # corrX: dx8 = e4m3(dx * 2^16) (dx ~ 2^-11 x)
    SC_W8 = np.float32(2.0 ** 9)    # corrX: W8  = e4m3(W * 2^9)
else:
    SC_DX = np.float32(2.0 ** 17)   # corrX: dx8 = e4m3(dx * 2^17) (dx ~ 2^-12 x)
    SC_W8 = np.float32(2.0 ** 8)    # corrX: W8  = e4m3(W * 2^8)
SC_X8 = np.float32(2.0 ** 5)    # fp8 corrW: x8  = e4m3(x * 2^5)
SC_DW = np.float32(2.0 ** 20)   # fp8 corrW: dW8 = e4m3(dW * 2^20)
INV_PROD = np.float32(2.0 ** -25)

LAST_RESULTS = None       # BassKernelResults of the most recent run (for profiling)
_NC = None                # cached compiled Bass module (build once per process)


def _round_sig12(a: np.ndarray) -> np.ndarray:
    """Round fp32 magnitudes to 12-bit significands (11 explicit mantissa
    bits), round-half-to-even. Values of this form pass through the PE's
    float32r (fp22) operand read exactly."""
    u = a.view(np.uint32).astype(np.uint64)
    half = np.uint64(1 << 11)
    one = np.uint64(1)
    r = (u + half - one + ((u >> np.uint64(12)) & one)) & ~np.uint64((1 << 12) - 1)
    return r.astype(np.uint32).view(np.float32)


def _build():
    nc = bacc.Bacc(num_devices=NCORES)
    wdt = dt.float16 if USE_FP16_MAIN else dt.float32r
    xdt = dt.float16 if USE_FP16_MAIN else dt.float32r
    xh = nc.dram_tensor("xh", [P, KO, BS], xdt, kind="ExternalInput")
    xc = nc.dram_tensor("xc", [P, KO, 2, BS], dt.float8e4, kind="ExternalInput")
    w0h = nc.dram_tensor("w0h", [P, KO, D], wdt, kind="ExternalInput")
    wc = nc.dram_tensor("wc", [P, KO, 2, D], dt.float8e4, kind="ExternalInput")
    w1 = nc.dram_tensor("w1", [P, KO, D], dt.float8e4, kind="ExternalInput")
    w2 = nc.dram_tensor("w2", [P, KO, D], dt.float8e4, kind="ExternalInput")
    vec = nc.dram_tensor("vec", [P, 6, KO], dt.float32, kind="ExternalInput")
    out = nc.dram_tensor("out", [P, KO, BS], dt.float32, kind="ExternalOutput")

    Sign = mybir.ActivationFunctionType.Sign
    Ident = mybir.ActivationFunctionType.Identity
    DR = mybir.MatmulPerfMode.DoubleRow
    ts = bass.ts

    with tile.TileContext(nc) as tc:
        with (
            tc.tile_pool(name="wpool", bufs=1) as wpool,
            tc.tile_pool(name="xpool", bufs=2) as xpool,
            tc.tile_pool(name="spool", bufs=2) as spool,
            tc.tile_pool(name="opool", bufs=4) as opool,
            tc.tile_pool(name="pspool", bufs=8, space="PSUM") as pspool,
        ):
            w0h_sb = wpool.tile([P, KO, D], wdt)
            wc_sb = wpool.tile([P, KO, 2, D], dt.float8e4)
            w1_sb = wpool.tile([P, KO, D], dt.float8e4)
            w2_sb = wpool.tile([P, KO, D], dt.float8e4)
            vec_sb = wpool.tile([P, 6, KO], dt.float32)

            xh_t, xc_t = xh[:], xc[:]
            w0h_t, wc_t = w0h[:], wc[:]
            out_t = out[:]

            # deadline-ordered DMA (single queue, ~220GB/s early): bt0 x,
            # main weights (per m-block), BN vector, bt0 corrections,
            # correction weights, L1/L2 weights (needed ~55/62us), then the
            # bt1 prefetch.
            x_tiles = []
            for bt in (0, 1):
                sl = bass.ds(bt * BT, BT)
                xh_sb = xpool.tile([P, KO, BT], xdt, tag="xh")
                xc_sb = xpool.tile([P, KO, 2, BT], dt.float8e4, tag="xc")
                x_tiles.append((xh_sb, xc_sb))
                if bt == 0:
                    nc.sync.dma_start(xh_sb, xh_t[:, :, sl])
                    for m in range(KO):
                        msl = ts(m, P)
                        nc.sync.dma_start(w0h_sb[:, :, msl], w0h_t[:, :, msl])
                    nc.sync.dma_start(vec_sb, vec[:])
                    nc.sync.dma_start(xc_sb, xc_t[:, :, :, sl])
                    for m in range(KO):
                        msl = ts(m, P)
                        nc.sync.dma_start(wc_sb[:, :, :, msl], wc_t[:, :, :, msl])
                    nc.sync.dma_start(w1_sb, w1[:])
                    nc.sync.dma_start(w2_sb, w2[:])
                else:
                    nc.sync.dma_start(xh_sb, xh_t[:, :, sl])
                    nc.sync.dma_start(xc_sb, xc_t[:, :, :, sl])

            def emit_tile(xh_sb, xc_sb, bt, c0, cols, sync_out=False):
                """One batch tile's compute: phase-split L0, then L1, L2.
                c0/cols select a column range within the SBUF x tiles (the
                last batch tile runs as two halves)."""
                cs = bass.ds(c0, cols)
                osl = bass.ds(bt * BT + c0, cols)
                # L0 in two groups of 4 m-blocks: all f32r mains of the
                # group, then its fp8-DR corrections — the f32r->fp8 PE
                # transition costs ~165ns, so batching same-dtype matmuls
                # pays, while holding only 4 PSUM banks keeps slack.
                s0_sb = spool.tile([P, KO, BT], dt.float8e4, tag="s0")
                GRP = 8 if USE_FP16_MAIN else 1
                for g in range(0, KO, GRP):
                    ps_g = [pspool.tile([P, BT], dt.float32, tag="ps",
                                        name=f"ps0_{g + i}")
                            for i in range(GRP)]
                    for i in range(GRP):
                        for k in range(KO):
                            nc.tensor.matmul(ps_g[i][:, :cols],
                                             w0h_sb[:, k, ts(g + i, P)],
                                             xh_sb[:, k, cs],
                                             start=k == 0, stop=False)
                    for i in range(GRP):
                        m = g + i
                        for k in range(KO):
                            nc.tensor.matmul(ps_g[i][:, :cols],
                                             wc_sb[:, k, :, ts(m, P)],
                                             xc_sb[:, k, :, cs],
                                             start=False, stop=k == KO - 1,
                                             perf_mode=DR)
                        nc.scalar.activation(s0_sb[:, m, cs], ps_g[i][:, :cols],
                                             Sign,
                                             bias=vec_sb[:, 1, m:m + 1],
                                             scale=vec_sb[:, 0, m:m + 1])
                # L1
                s1_sb = spool.tile([P, KO, BT], dt.float8e4, tag="s1")
                for m in range(KO):
                    ps = pspool.tile([P, BT], dt.float32, tag="ps")
                    for kp in range(KO // 2):
                        nc.tensor.matmul(ps[:, :cols],
                                         w1_sb[:, 2 * kp:2 * kp + 2, ts(m, P)],
                                         s0_sb[:, 2 * kp:2 * kp + 2, cs],
                                         start=kp == 0, stop=kp == KO // 2 - 1,
                                         perf_mode=DR)
                    nc.scalar.activation(s1_sb[:, m, cs], ps[:, :cols], Sign,
                                         bias=vec_sb[:, 3, m:m + 1],
                                         scale=vec_sb[:, 2, m:m + 1])
                # L2
                for m in range(KO):
                    ps = pspool.tile([P, BT], dt.float32, tag="ps")
                    for kp in range(KO // 2):
                        nc.tensor.matmul(ps[:, :cols],
                                         w2_sb[:, 2 * kp:2 * kp + 2, ts(m, P)],
                                         s1_sb[:, 2 * kp:2 * kp + 2, cs],
                                         start=kp == 0, stop=kp == KO // 2 - 1,
                                         perf_mode=DR)
                    o_sb = opool.tile([P, BT], dt.float32, tag="om")
                    nc.scalar.activation(o_sb[:, :cols], ps[:, :cols], Ident,
                                         bias=vec_sb[:, 5, m:m + 1],
                                         scale=vec_sb[:, 4, m:m + 1])
                    # out on the Scalar engine's DMA queue: keeps the Sync
                    # queue a pure input stream (no compute-dependent DMA
                    # blocking the next tile's prefetch behind it), and the
                    # in-order scalar queue sequences it right after its ACT.
                    # Final tile: Sync is idle (all inputs loaded), and its
                    # queue drains the tail without blocking the last ACTs.
                    eng = nc.sync if sync_out else nc.scalar
                    eng.dma_start(out_t[:, m, osl], o_sb[:, :cols])

            for bt in range(NBT):
                sl = bass.ds(bt * BT, BT)
                if bt < 2:
                    xh_sb, xc_sb = x_tiles[bt]
                else:
                    xh_sb = xpool.tile([P, KO, BT], xdt, tag="xh")
                    xc_sb = xpool.tile([P, KO, 2, BT], dt.float8e4, tag="xc")
                    nc.sync.dma_start(xh_sb, xh_t[:, :, sl])
                    nc.sync.dma_start(xc_sb, xc_t[:, :, :, sl])
                if bt == NBT - 1:
                    # split the final tile to shorten the post-matmul tail
                    emit_tile(xh_sb, xc_sb, bt, 0, BT // 2, sync_out=True)
                    emit_tile(xh_sb, xc_sb, bt, BT // 2, BT // 2,
                              sync_out=True)
                else:
                    emit_tile(xh_sb, xc_sb, bt, 0, BT)

    nc.compile()
    return nc


def kernel(**inputs) -> np.ndarray:
    global LAST_RESULTS
    f32 = np.float32
    e4m3 = mybir.dt.np(dt.float8e4)
    x = np.asarray(inputs["x"], f32)
    W0 = np.asarray(inputs["W0"], f32)
    b0 = np.asarray(inputs["b0"], f32)
    W1 = np.asarray(inputs["W1"], f32)
    b1 = np.asarray(inputs["b1"], f32)
    W2 = np.asarray(inputs["W2"], f32)
    b2 = np.asarray(inputs["b2"], f32)
    bn0_g = np.asarray(inputs["bn0_g"], f32)
    bn0_b = np.asarray(inputs["bn0_b"], f32)
    bn0_rm = np.asarray(inputs["bn0_rm"], f32)
    bn0_rv = np.asarray(inputs["bn0_rv"], f32)
    bn1_g = np.asarray(inputs["bn1_g"], f32)
    bn1_b = np.asarray(inputs["bn1_b"], f32)
    bn1_rm = np.asarray(inputs["bn1_rm"], f32)
    bn1_rv = np.asarray(inputs["bn1_rv"], f32)
    osc = np.asarray(inputs["out_scale"], f32)

    # per-channel affine folds (BN in eval mode):
    #   bn0(h+b0) = h*A0 + B0 ; bn1(h+b1) = h*A1 + B1 ; out = h*CS + CB
    # L0's psum carries h0*2^25, so A0 absorbs the 2^-25.
    inv0 = (bn0_g / np.sqrt(bn0_rv + EPS)).astype(f32)
    inv1 = (bn1_g / np.sqrt(bn1_rv + EPS)).astype(f32)
    A0 = (inv0 * INV_PROD).astype(f32)
    B0 = ((b0 - bn0_rm) * inv0 + bn0_b).astype(f32)
    A1, B1 = inv1, ((b1 - bn1_rm) * inv1 + bn1_b).astype(f32)
    CS, CB = osc, (b2 * osc).astype(f32)
    vec = np.stack([A0, B0, A1, B1, CS, CB])           # [6, D]
    vec_host = np.ascontiguousarray(
        vec.reshape(6, KO, P).transpose(2, 0, 1))      # [P, 6, KO]

    def pm(a):
        # [cols, D] -> partition-major [P, KO, cols]
        return np.ascontiguousarray(a.T.reshape(KO, P, -1).transpose(1, 0, 2))

    if USE_FP16_MAIN:
        Wh_sc = (W0 * SC_WH).astype(np.float16)        # stationary (2^12)
        dW = (W0 - Wh_sc.astype(f32) / SC_WH).astype(f32)
        w0h_host = pm(Wh_sc)
    else:
        W0h = _round_sig12(W0)
        dW = W0 - W0h
        w0h_host = pm(W0h * SC_WH)
    W8 = (W0 * SC_W8).astype(e4m3)
    dW8 = (dW * SC_DW).astype(e4m3)
    # correction weight pairs: [P, KO, 2, D]; slot0 = W8 (corrX, pairs with
    # dx8), slot1 = dW8 (corrW, pairs with x8)
    wc_host = np.ascontiguousarray(
        np.stack([pm(W8), pm(dW8)], axis=2))
    w1_host = pm(np.sign(W1).astype(e4m3))
    w2_host = pm(np.sign(W2).astype(e4m3))

    if USE_FP16_MAIN:
        xh_sc = (x * SC_XH).astype(np.float16)      # [*, D] fp16 (2^13)
        dx = (x - xh_sc.astype(f32) / SC_XH).astype(f32)
        xhT = pm(xh_sc)
    else:
        xh_full = _round_sig12(x)
        dx = x - xh_full
        xhT = pm(xh_full * SC_XH)                   # [P, KO, B] f32r (2^13)
    dx8 = (dx * SC_DX).astype(e4m3)
    x8 = (x * SC_X8).astype(e4m3)
    xcT = np.ascontiguousarray(
        np.stack([pm(dx8), pm(x8)], axis=2))        # [P, KO, 2, B]

    shared = {
        "w0h": w0h_host, "wc": wc_host,
        "w1": w1_host, "w2": w2_host, "vec": vec_host,
    }
    in_maps = []
    for c in range(NCORES):
        bs = slice(c * BS, (c + 1) * BS)
        in_maps.append({
            **shared,
            "xh": np.ascontiguousarray(xhT[:, :, bs]),
            "xc": np.ascontiguousarray(xcT[:, :, :, bs]),
        })

    global _NC
    if _NC is None:
        _NC = _build()
    res = run_bass_kernel_spmd(_NC, in_maps, core_ids=list(range(NCORES)))
    LAST_RESULTS = res

    out = np.empty((B, D), f32)
    for c in range(NCORES):
        # [P, KO, BS] -> [BS, KO*P] with channel = ko*P + p
        o = res.results[c]["out"].transpose(2, 1, 0).reshape(BS, D)
        out[c * BS:(c + 1) * BS] = o
    return out
